# revision 35
# baseline (speedup 1.0000x reference)
"""EnhancedCrossAttention on 8 Trainium2 NeuronCores.

Sharding: core c = 4*b + g handles batch b and head-group g (4 of 16 heads).
Wq/Wk/Wv split column-wise per head group, Wo row-wise; partial outputs
summed on host (tensor-parallel allreduce done at gather time).

Per-core device kernel (all matmuls fp32r = e8m11, full PE rate):
  P1: QpT[256,1024]  = (Wq_g * scale)^T-projection, transposed layout
  P2: per kv-chunk(512): KpT[256,512] proj (transposed), Vp[512,260] proj
      (natural layout, 4 heads x (64 cols + ones col))
  P3: per chunk/head/kv-tile: S^T[128kv,1024q] = K Q^T; P^T = exp(obj*S^T)
      (obj rides the ACT per-partition scale); O^T[65,1024] += [V|1]^T P^T
      accumulated in PSUM per chunk, DVE-added into SBUF across chunks
      (row 64 = softmax denominator l)
  P4: per head: recip(l) -> PE ones-outer-product broadcast -> normalize;
      Y[1024,1024] = O^T.T @ Wo_g, DMA out.
"""

import numpy as np

DIM = 1024
H = 16
HD = 64
B = 2
NQ = 1024
NKV = 4096
HPG = 4           # heads per group (per core)
DH = HPG * HD     # 256 head-dim columns per core
NCORES = 8
KV_CHUNK = 512
N_CHUNKS = NKV // KV_CHUNK
KT = DIM // 128   # k-tiles over DIM

_prog_cache = {}


def _build(has_bq, has_bk, has_bv):
    key = (has_bq, has_bk, has_bv)
    if key in _prog_cache:
        return _prog_cache[key]

    import concourse.mybir as mybir
    import concourse.tile as tile
    from concourse import bacc

    f32 = mybir.dt.float32
    f32r = mybir.dt.float32r
    EXP = mybir.ActivationFunctionType.Exp
    MULT = mybir.AluOpType.mult

    nc = bacc.Bacc("TRN2")
    xqt = nc.dram_tensor("xqt", [DIM, NQ], f32r, kind="ExternalInput")
    xkt = nc.dram_tensor("xkt", [DIM, NKV], f32r, kind="ExternalInput")
    xvt = nc.dram_tensor("xvt", [DIM, NKV], f32r, kind="ExternalInput")
    wq = nc.dram_tensor("wq", [DIM, DH], f32r, kind="ExternalInput")
    wk = nc.dram_tensor("wk", [DIM, DH], f32r, kind="ExternalInput")
    wv = nc.dram_tensor("wv", [DIM, DH], f32r, kind="ExternalInput")
    wo = nc.dram_tensor("wo", [DH, DIM], f32r, kind="ExternalInput")
    obj = nc.dram_tensor("obj", [NKV], f32, kind="ExternalInput")
    bq = nc.dram_tensor("bq", [DH], f32, kind="ExternalInput") if has_bq else None
    bk = nc.dram_tensor("bk", [DH], f32, kind="ExternalInput") if has_bk else None
    bv = nc.dram_tensor("bv", [DH], f32r, kind="ExternalInput") if has_bv else None
    y = nc.dram_tensor("y", [NQ, DIM], f32, kind="ExternalOutput")

    with tile.TileContext(nc) as tc:
        with tc.tile_pool(name="const", bufs=1) as cpool:
            wq_sb = cpool.tile([128, KT, DH], f32r, tag="wq")
            wk_sb = cpool.tile([128, KT, DH], f32r, tag="wk")
            wv_sb = cpool.tile([128, KT, DH], f32r, tag="wv")
            wo_sb = cpool.tile([128, DH // 128, DIM], f32r, tag="wo")
            obj_sb = cpool.tile([128, NKV // 128], f32, tag="obj")
            ones_sb = cpool.tile([128, 128], f32r, tag="ones")
            qpt = cpool.tile([128, 2, NQ], f32r, tag="qpt")
            ot = cpool.tile([128, 2, NQ], f32r, tag="ot")
            oacc = [
                cpool.tile([65, NQ], f32, tag=f"oacc{h}", name=f"oacc{h}")
                for h in range(HPG)
            ]

            # DMA emission order matters for startup latency: wq first (P1
            # stationary), then per-k xq slices (P1 moving, pipelined), then
            # the rest in order of first use. wo is only needed at P4.
            nc.sync.dma_start(wq_sb[:], wq[:].rearrange("(k p) n -> p k n", p=128))
            nc.gpsimd.memset(ones_sb[:].bitcast(f32), 1.0)
            if has_bq:
                bq_sb = cpool.tile([128, 2], f32, tag="bq")
                nc.sync.dma_start(bq_sb[:], bq[:].rearrange("(m p) -> p m", p=128))
            if has_bk:
                bk_sb = cpool.tile([128, 2], f32, tag="bk")
                nc.sync.dma_start(bk_sb[:], bk[:].rearrange("(m p) -> p m", p=128))
            if has_bv:
                bv_sb = cpool.tile([1, DH], f32r, tag="bv")
                nc.sync.dma_start(bv_sb[:], bv[:].rearrange("(a n) -> a n", a=1))

            with (
                tc.tile_pool(name="pj", bufs=2, space="PSUM") as pjpool,
                tc.tile_pool(name="spsum", bufs=2, space="PSUM") as spool,
                tc.tile_pool(name="opsum", bufs=1, space="PSUM") as opool,
            ):
                # ---- P1: Q projection (transposed out) ----
                with tc.tile_pool(name="xq", bufs=1) as xqpool:
                    xq_sb = xqpool.tile([128, KT, NQ], f32r, tag="xq")
                    xq_re = xqt[:].rearrange("(k p) n -> p k n", p=128)
                    for k in range(KT):
                        nc.sync.dma_start(xq_sb[:, k, :], xq_re[:, k, :])
                    # wk right after xq: needed with chunk-0 xk for K
                    # proj; per-k slices so the first K-proj matmul only
                    # waits for 128KB of it
                    wk_re = wk[:].rearrange("(k p) n -> p k n", p=128)
                    for k in range(KT):
                        nc.sync.dma_start(wk_sb[:, k, :], wk_re[:, k, :])
                    for m in range(2):
                        for n in range(2):
                            ps = pjpool.tile([128, 512], f32, tag="pj")
                            for k in range(KT):
                                nc.tensor.matmul(
                                    ps[:],
                                    wq_sb[:, k, m * 128 : (m + 1) * 128],
                                    xq_sb[:, k, n * 512 : (n + 1) * 512],
                                    start=(k == 0),
                                    stop=(k == KT - 1),
                                )
                            dst = qpt[:, m, n * 512 : (n + 1) * 512]
                            if has_bq:
                                nc.vector.tensor_scalar_add(
                                    dst, ps[:], bq_sb[:, m : m + 1]
                                )
                            else:
                                nc.vector.tensor_copy(dst, ps[:])

                # ---- P2+P3: stream kv chunks; project K/V; attention ----
                with (
                    tc.tile_pool(name="xk", bufs=2) as xkpool,
                    tc.tile_pool(name="xv", bufs=2) as xvpool,
                    tc.tile_pool(name="kpt", bufs=2) as kptpool,
                    tc.tile_pool(name="vp", bufs=2) as vppool,
                    tc.tile_pool(name="pt", bufs=3) as ptpool,
                    tc.tile_pool(name="norm", bufs=2) as npool,
                ):
                    def normalize_head(h):
                        # recip of l (row 64) -> PE broadcast over 64
                        # partitions -> normalized O_h^T into ot
                        m = h // 2
                        rec = npool.tile([65, NQ], f32r, tag="rec", name="rec")
                        with nc.allow_low_precision("softmax recip rounding"):
                            nc.vector.reciprocal(rec[64:65, :], oacc[h][64:65, :])
                        otmp = None
                        if h % 2 == 1:
                            otmp = npool.tile([64, NQ], f32r, tag="otmp", name="otmp")
                        for n in range(2):
                            nsl = slice(n * 512, (n + 1) * 512)
                            psr = pjpool.tile([128, 512], f32, tag="pj", name="psr")
                            nc.tensor.matmul(
                                psr[0:64, :],
                                ones_sb[64:65, 0:64],
                                rec[64:65, nsl],
                                start=True,
                                stop=True,
                            )
                            dst = otmp[:, nsl] if h % 2 else ot[0:64, m, nsl]
                            nc.vector.tensor_tensor(
                                dst, oacc[h][0:64, nsl], psr[0:64, :], MULT
                            )
                        if h % 2 == 1:
                            for n in range(2):
                                nsl = slice(n * 512, (n + 1) * 512)
                                nc.sync.dma_start(ot[64:128, m, nsl], otmp[:, nsl])

                    xk_re = xkt[:].rearrange("(k p) n -> p k n", p=128)
                    xv_re = xvt[:].rearrange("(k p) n -> p k n", p=128)

                    def load_k(c):
                        cs = slice(c * KV_CHUNK, (c + 1) * KV_CHUNK)
                        xk_c = xkpool.tile(
                            [128, KT, KV_CHUNK], f32r, tag="xk", name="xk_c"
                        )
                        for k in range(KT):
                            nc.sync.dma_start(xk_c[:, k, :], xk_re[:, k, cs])
                        return xk_c

                    def load_v(c):
                        cs = slice(c * KV_CHUNK, (c + 1) * KV_CHUNK)
                        xv_c = xvpool.tile(
                            [128, KT, KV_CHUNK], f32r, tag="xv", name="xv_c"
                        )
                        for k in range(KT):
                            nc.sync.dma_start(xv_c[:, k, :], xv_re[:, k, cs])
                        return xv_c

                    ps_o = {}
                    prefetched = {0: (load_k(0), None)}
                    # consts not needed until after chunk-0 K-proj begins
                    nc.sync.dma_start(
                        wv_sb[:], wv[:].rearrange("(k p) n -> p k n", p=128)
                    )
                    nc.sync.dma_start(
                        obj_sb[:], obj[:].rearrange("(t p) -> p t", p=128)
                    )
                    for c in range(N_CHUNKS):
                        if c in prefetched:
                            xk_c, xv_c = prefetched.pop(c)
                        else:
                            xk_c, xv_c = load_k(c), load_v(c)
                        if c == 5:
                            # wo for P4: load in the tail of the streaming
                            # phase when DMA has spare bandwidth
                            nc.sync.dma_start(
                                wo_sb[:],
                                wo[:].rearrange("(t p) n -> p t n", p=128),
                            )
                        # K^T projection for this chunk
                        kpt_c = kptpool.tile([128, 2, KV_CHUNK], f32r, tag="kpt")
                        for m in range(2):
                            ps = pjpool.tile([128, 512], f32, tag="pj")
                            for k in range(KT):
                                nc.tensor.matmul(
                                    ps[:],
                                    wk_sb[:, k, m * 128 : (m + 1) * 128],
                                    xk_c[:, k, :],
                                    start=(k == 0),
                                    stop=(k == KT - 1),
                                )
                            if has_bk:
                                nc.vector.tensor_scalar_add(
                                    kpt_c[:, m, :], ps[:], bk_sb[:, m : m + 1]
                                )
                            else:
                                nc.vector.tensor_copy(kpt_c[:, m, :], ps[:])
                        if xv_c is None:
                            xv_c = load_v(c)
                        # V projection (natural layout + ones columns)
                        vp_c = vppool.tile([128, 4, HPG * 65], f32r, tag="vp")
                        nc.gpsimd.memset(vp_c[:].bitcast(f32), 1.0)
                        for t in range(4):
                            ps = pjpool.tile([128, 512], f32, tag="pj")
                            psv = ps[:, 0:DH]
                            for k in range(KT):
                                nc.tensor.matmul(
                                    psv,
                                    xv_c[:, k, t * 128 : (t + 1) * 128],
                                    wv_sb[:, k, :],
                                    start=(k == 0),
                                    stop=(k == KT - 1 and not has_bv),
                                )
                            if has_bv:
                                nc.tensor.matmul(
                                    psv,
                                    ones_sb[0:1, 0:128],
                                    bv_sb[0:1, :],
                                    start=False,
                                    stop=True,
                                )
                            nc.vector.tensor_copy(
                                vp_c[:, t, :].rearrange("p (h e) -> p h e", h=HPG)[
                                    :, :, 0:HD
                                ],
                                psv.rearrange("p (h e) -> p h e", h=HPG),
                            )
                        # attention on this chunk; in the last chunk run
                        # heads 3,2,1,0 so kt=1's heads normalize first (they
                        # feed the Y start-group below) and the critical last
                        # head is even (no cross-partition DMA hop)
                        h_order = (
                            [3, 2, 1, 0] if c == N_CHUNKS - 1 else range(HPG)
                        )
                        for h in h_order:
                            hb = (h % 2) * 64
                            m = h // 2
                            ps_o[h] = opool.tile(
                                [65, NQ], f32, tag="o", name=f"ps_o{h}"
                            )
                            for t in range(4):
                                ps_s = spool.tile([128, NQ], f32, tag="s")
                                for n in range(2):
                                    nc.tensor.matmul(
                                        ps_s[:, n * 512 : (n + 1) * 512],
                                        kpt_c[hb : hb + 64, m, t * 128 : (t + 1) * 128],
                                        qpt[hb : hb + 64, m, n * 512 : (n + 1) * 512],
                                        start=True,
                                        stop=True,
                                    )
                                pt_t = ptpool.tile([128, NQ], f32r, tag="pt")
                                ti = c * 4 + t
                                nc.scalar.activation(
                                    pt_t[:], ps_s[:], EXP,
                                    scale=obj_sb[:, ti : ti + 1],
                                )
                                for n in range(2):
                                    nc.tensor.matmul(
                                        ps_o[h][:, n * 512 : (n + 1) * 512],
                                        vp_c[:, t, h * 65 : (h + 1) * 65],
                                        pt_t[:, n * 512 : (n + 1) * 512],
                                        start=(t == 0),
                                        stop=(t == 3),
                                    )
                            if c == 0:
                                nc.vector.tensor_copy(oacc[h][:], ps_o[h][:])
                            else:
                                nc.vector.tensor_add(
                                    oacc[h][:], oacc[h][:], ps_o[h][:]
                                )
                            if c == N_CHUNKS - 1:
                                normalize_head(h)


                    # ---- P4b: Y = O^T.T @ Wo (reuses s psum) ----
                    with tc.tile_pool(name="yb", bufs=3) as ypool:
                        for mq in range(NQ // 128):
                            psy = spool.tile([128, NQ], f32, tag="s", name="psy")
                            for kt2 in (1, 0):
                                for n in range(2):
                                    nc.tensor.matmul(
                                        psy[:, n * 512 : (n + 1) * 512],
                                        ot[:, kt2, mq * 128 : (mq + 1) * 128],
                                        wo_sb[:, kt2, n * 512 : (n + 1) * 512],
                                        start=(kt2 == 1),
                                        stop=(kt2 == 0),
                                    )
                            yt = ypool.tile([128, NQ], f32, tag="yt")
                            nc.scalar.copy(yt[:], psy[:])
                            nc.sync.dma_start(
                                y[mq * 128 : (mq + 1) * 128, :], yt[:]
                            )

    nc.compile()
    _prog_cache[key] = nc
    return nc


def kernel(query, key, value, objectness_scores, Wq, bq, Wk, bk, Wv, bv, Wo, bo,
           _trace=False):
    from concourse.bass_utils import run_bass_kernel_spmd

    f = np.float32
    query = np.asarray(query, f)
    key_ = np.asarray(key, f)
    value = np.asarray(value, f)
    objs = np.asarray(objectness_scores, f)
    Wq = np.asarray(Wq, f); bq = np.asarray(bq, f)
    Wk = np.asarray(Wk, f); bk = np.asarray(bk, f)
    Wv = np.asarray(Wv, f); bv = np.asarray(bv, f)
    Wo = np.asarray(Wo, f); bo = np.asarray(bo, f)

    scale = np.float32(HD ** -0.5)
    has_bq = bool(np.any(bq)); has_bk = bool(np.any(bk)); has_bv = bool(np.any(bv))
    nc = _build(has_bq, has_bk, has_bv)

    in_maps = []
    for c in range(NCORES):
        b, g = divmod(c, NCORES // B)
        sl = slice(g * DH, (g + 1) * DH)
        m = {
            "xqt": np.ascontiguousarray(query[b].T),
            "xkt": np.ascontiguousarray(key_[b].T),
            "xvt": np.ascontiguousarray(value[b].T),
            "wq": np.ascontiguousarray(Wq[:, sl] * scale),
            "wk": np.ascontiguousarray(Wk[:, sl]),
            "wv": np.ascontiguousarray(Wv[:, sl]),
            "wo": np.ascontiguousarray(Wo[sl, :]),
            "obj": np.ascontiguousarray(objs[b]),
        }
        if has_bq:
            m["bq"] = np.ascontiguousarray(bq[sl] * scale)
        if has_bk:
            m["bk"] = np.ascontiguousarray(bk[sl])
        if has_bv:
            m["bv"] = np.ascontiguousarray(bv[sl])
        in_maps.append(m)

    res = run_bass_kernel_spmd(
        nc, in_maps, core_ids=list(range(NCORES)), trace=_trace
    )
    out = np.zeros((B, NQ, DIM), np.float64)
    for c in range(NCORES):
        out[c // (NCORES // B)] += res.results[c]["y"].astype(np.float64)
    out += bo.astype(np.float64)
    result = out.astype(np.float32)
    if _trace:
        return result, res
    return result
